# revision 1
# baseline (speedup 1.0000x reference)
"""Trainium2 Bass kernel for nn_Capsule_16484084482446.

Reference math collapses: with cw = softmax(rw, axis=1),
  outputs[b,j,d] = sum_i sum_n cw[b,i,n] * u[b,j,n,d]
                 = sum_n u[b,j,n,d]           (since sum_i cw[b,i,n] == 1)
so the routing loop is a no-op and the final result is
  out = (sum_n x[b,n,:]) @ W   reshaped to (B, 10, 16).

Kernel strategy (data-parallel over batch, 4 batches per core x 8 cores):
  per core: x_shard (4, 4096, 128) viewed as 128 partitions x (128 rows x 128 d);
  partition p holds rows [128p, 128p+128), so batch b owns partitions [32b, 32b+32).
  1. Staggered chunked HWDGE DMAs (small chunks first so VectorE starts early).
  2. VectorE folds each chunk's rows with in-place contiguous halving adds
     (measured ~1 cycle/elem vs ~1.7 for strided reduces) -> red_c (128, 128).
  3. PE accumulates every red_c into PSUM via a 0/1 batch-mask matmul
     -> s[d, b] = sum_p acc[p, d] * mask[p, b], overlapped with VectorE.
  4. PE matmul s^T @ W -> (4, 160) per-core output.

Raw Bass (no TileContext): Tile's tail drain needs more sync-wait slots than the
TRN2 CTRL encoding allows for this DMA-lane mix, and its end-of-kernel barriers
would dominate a ~40 us kernel. Every semaphore is cleared by its final consumer
right after its last wait, so the NEFF re-executes cleanly (profilers loop it).
"""

from contextlib import ExitStack

import numpy as np

import concourse.bass as bass
from concourse import mybir
from concourse.bass_utils import run_bass_kernel_spmd

N_CORES = 8
B, N, DIN = 32, 4096, 128
BSH = B // N_CORES          # 4 batches per core
DOUT = 160                  # 10 capsules * 16 dims
# rows-per-partition split; geometric ramp (early VectorE start), steady
# middle, small last (tiny final fold after the last DMA lands)
CHUNKS = [4, 8, 16, 16, 16, 16, 16, 16, 16, 4]
# max DMAs in flight before throttling issue against VectorE fold progress
# (len(CHUNKS) = unthrottled; measured best — throttling lowered aggregate
# DMA bandwidth more than it helped chunk-arrival latency)
DMA_FLIGHT = len(CHUNKS)
assert sum(CHUNKS) == BSH * N // 128
NCHUNK = len(CHUNKS)

F32 = mybir.dt.float32

_cache = {}


def _build_nc(intra_dve_sems=False, clears=True, chunks=None, flight=None):
    """intra_dve_sems: add same-engine RAW semaphores between the in-place
    halving adds. The DVE drains its pipe between ops so hardware doesn't
    need them; CoreSim's race checker does."""
    global CHUNKS, NCHUNK, DMA_FLIGHT
    if chunks is not None:
        CHUNKS = chunks
        NCHUNK = len(CHUNKS)
    if flight is not None:
        DMA_FLIGHT = flight
    assert sum(CHUNKS) == BSH * N // 128
    nc = bass.Bass()
    x = nc.dram_tensor("x", [BSH, N, DIN], F32, kind="ExternalInput")
    w = nc.dram_tensor("W", [DIN, DOUT], F32, kind="ExternalInput")
    out = nc.dram_tensor("out", [BSH, DOUT], F32, kind="ExternalOutput")

    # (128, 128, 128): partition p, row-in-partition n, feature d
    x3 = x[:].flatten_outer_dims().rearrange("(p n) d -> p n d", p=128)
    starts = np.cumsum([0] + CHUNKS).tolist()

    with ExitStack() as ctx:
        ec = ctx.enter_context
        xc = [ec(nc.sbuf_tensor(f"xc{c}", [128, CHUNKS[c] * DIN], F32))
              for c in range(NCHUNK)]
        w_sb = ec(nc.sbuf_tensor("w_sb", [DIN, DOUT], F32))
        mask_sb = ec(nc.sbuf_tensor("mask_sb", [128, BSH], F32))
        s_sb = ec(nc.sbuf_tensor("s_sb", [DIN, BSH], F32))
        out_sb = ec(nc.sbuf_tensor("out_sb", [BSH, DOUT], F32))
        psum_s = ec(nc.psum_tensor("psum_s", [DIN, BSH], F32))
        psum_o = ec(nc.psum_tensor("psum_o", [BSH, DOUT], F32))

        dma_w = ec(nc.semaphore("dma_w"))
        dma_c = [ec(nc.semaphore(f"dma_c{c}")) for c in range(NCHUNK)]
        v_red = ec(nc.semaphore("v_red"))    # +1 per finished red_c
        v_chain = ec(nc.semaphore("v_chain"))  # intra-DVE RAW links (sim only)
        pe_sem = ec(nc.semaphore("pe_sem"))
        v_sem = ec(nc.semaphore("v_sem"))    # s_sb ready
        v_out = ec(nc.semaphore("v_out"))
        dma_out = ec(nc.semaphore("dma_out"))
        # Sem hygiene without an entry barrier: every semaphore is cleared by
        # its final consumer right after the consumer's last wait on it, so
        # every run (the profiler re-executes the NEFF) starts from zeros.
        block = ec(nc.Block())

        @block.sync
        def _(sync):
            for c in range(NCHUNK):
                if c >= DMA_FLIGHT:
                    # flow control against VectorE's fold progress (v_red),
                    # not against dma_c — DVE clears dma_c right after its
                    # own wait, which would race a wait here
                    sync.wait_ge(v_red, c - DMA_FLIGHT + 1)
                sync.dma_start(
                    xc[c][:], x3[:, starts[c] : starts[c + 1], :]
                ).then_inc(dma_c[c], 16)
            # W is only needed for the final tiny matmul — load it last
            sync.dma_start(w_sb[:], w[:]).then_inc(dma_w, 16)
            sync.wait_ge(v_out, 1)
            if clears:
                sync.sem_clear(v_out)
            sync.dma_start(out[:], out_sb[:]).then_inc(dma_out, 16)
            sync.wait_ge(dma_out, 16)
            if clears:
                sync.sem_clear(dma_out)

        @block.vector
        def _(vector):
            # 0/1 batch mask, one 32-partition quadrant at a time (nonzero
            # partition bases only allow 32-partition windows; disjoint
            # pieces keep the sim's WAW checker happy)
            for q in range(4):
                for b in range(BSH):
                    vector.memset(
                        mask_sb[32 * q : 32 * (q + 1), b : b + 1],
                        1.0 if q == b else 0.0,
                    )
            links = 0
            for c in range(NCHUNK):
                vector.wait_ge(dma_c[c], 16)
                if clears:
                    vector.sem_clear(dma_c[c])
                t = xc[c]
                s = CHUNKS[c]
                while s > 1:
                    s //= 2
                    op = vector.tensor_add(
                        t[:, : s * DIN],
                        t[:, : s * DIN],
                        t[:, s * DIN : 2 * s * DIN],
                    )
                    if intra_dve_sems and s > 1:
                        op.then_inc(v_chain, 1)
                        links += 1
                        vector.wait_ge(v_chain, links)
                # red_c = t[:, :DIN] done; tell PE
                op.then_inc(v_red, 1)
            if intra_dve_sems and clears:
                vector.sem_clear(v_chain)
            vector.wait_ge(pe_sem, 1)
            vector.tensor_copy(s_sb[:], psum_s[:]).then_inc(v_sem, 1)
            vector.wait_ge(pe_sem, 2)
            if clears:
                vector.sem_clear(pe_sem)
            vector.tensor_copy(out_sb[:], psum_o[:]).then_inc(v_out, 1)

        @block.tensor
        def _(tensor):
            # s[d, b] += sum_p red_c[p, d] * mask[p, b], accumulated over chunks
            for c in range(NCHUNK):
                tensor.wait_ge(v_red, c + 1)
                mm = tensor.matmul(
                    psum_s[:],
                    xc[c][:, :DIN],
                    mask_sb[:],
                    start=(c == 0),
                    stop=(c == NCHUNK - 1),
                )
            mm.then_inc(pe_sem, 1)
            if clears:
                tensor.sem_clear(v_red)
            tensor.wait_ge(dma_w, 16)
            if clears:
                tensor.sem_clear(dma_w)
            tensor.wait_ge(v_sem, 1)
            if clears:
                tensor.sem_clear(v_sem)
            # out[b, jd] = sum_d s[d, b] * W[d, jd]
            tensor.matmul(
                psum_o[:], s_sb[:], w_sb[:], start=True, stop=True
            ).then_inc(pe_sem, 1)

    return nc


def _get_nc():
    if "nc" not in _cache:
        _cache["nc"] = _build_nc()
    return _cache["nc"]


def _in_maps(x, W):
    x = np.ascontiguousarray(x, dtype=np.float32)
    W = np.ascontiguousarray(W, dtype=np.float32)
    return [{"x": x[i * BSH : (i + 1) * BSH], "W": W} for i in range(N_CORES)]


def kernel(x, W, **profile_kwargs):
    nc = _get_nc()
    res = run_bass_kernel_spmd(nc, _in_maps(x, W), list(range(N_CORES)), **profile_kwargs)
    out = np.concatenate([r["out"] for r in res.results], axis=0)
    ret = out.reshape(B, 10, 16).astype(np.float32)
    if profile_kwargs:
        ret = (ret, res)
    return ret



# revision 8
# speedup vs baseline: 1.1173x; 1.1173x over previous
"""Trainium2 Bass kernel for nn_Capsule_16484084482446.

Reference math collapses: with cw = softmax(rw, axis=1),
  outputs[b,j,d] = sum_i sum_n cw[b,i,n] * u[b,j,n,d]
                 = sum_n u[b,j,n,d]           (since sum_i cw[b,i,n] == 1)
so the routing loop is a no-op and the final result is
  out = (sum_n x[b,n,:]) @ W   reshaped to (B, 10, 16).

Kernel strategy (data-parallel over batch, 4 batches per core x 8 cores):
  per core: x_shard (4, 4096, 128) viewed as 128 partitions x (128 rows x 128 d);
  partition p holds rows [128p, 128p+128), so batch b owns partitions [32b, 32b+32).

Profile-driven structure (trace: x-stream runs at ~367 GB/s with zero gaps on a
single sync-ring DMA chain; all remaining time is tail + fixed NEFF overhead):
  1. Sync issues chunked HWDGE DMAs back-to-back; chunk sizes taper at the end
     ([...,8,8,4,4]) so the post-stream fold+matmul tail is short.
  2. VectorE folds each chunk with halving adds; the first add narrows fp32 ->
     bf16 (same DVE rate), later levels run bf16 in-place at 2x DVE rate.
  3. PE accumulates each chunk's bf16 red via a single-pass bf16 matmul against
     a 0/1 batch mask -> psum_s[d, b]  (fp32 LOW_HIGH would cost 2x LDW+MM).
  4. psum_s -> s_bf (bf16 cast copy), then one bf16 matmul s^T @ W_bf -> out.
     W loads on the otherwise-idle Scalar (ACT) HWDGE ring and Scalar itself
     casts it to bf16 — NOT via GpSimd SWDGE: a single SWDGE DMA makes SDMA
     engine 15 straggle ~4.5 us on the x-stream (descriptor-ring AXI port
     contention; measured). GpSimd only memsets the masks (off VectorE).
  bf16 only touches the tiny PE contractions (the 16384-row fold stays fp32 on
  DVE): measured rel err ~1e-3 vs the 2e-2 gate.

No in-kernel semaphore clears: the compiler-emitted NEFF epilogue clears every
kernel semaphore (S[3..255]) after each execution, so re-execution (profiler
loops the NEFF) always starts from zeros.

Raw Bass (no TileContext): Tile's tail drain needs more sync-wait slots than the
TRN2 CTRL encoding allows for this DMA-lane mix, and its end-of-kernel barriers
would dominate a ~35 us kernel.
"""

from contextlib import ExitStack

import numpy as np

import concourse.bass as bass
from concourse import mybir
from concourse.bass_utils import run_bass_kernel_spmd

N_CORES = 8
B, N, DIN = 32, 4096, 128
BSH = B // N_CORES          # 4 batches per core
DOUT = 160                  # 10 capsules * 16 dims
# rows-per-partition split; tapered tail so the last chunks' fold+matmul are
# tiny and the post-stream critical path stays short
CHUNKS = [8, 16, 16, 16, 16, 16, 16, 8, 8, 4, 4]
assert sum(CHUNKS) == BSH * N // 128

F32 = mybir.dt.float32
BF16 = mybir.dt.bfloat16

_cache = {}


def _build_nc(chunks=None, wait_out=False):
    """wait_out: wait for the output DMA's completion sem before the end-of-
    block barrier. The NEFF epilogue (engine barriers + 253 sem clears, ~7 us)
    runs after our last instruction either way, giving the 2.5 KB output write
    ample time to land before the runtime reads it back."""
    chunks = CHUNKS if chunks is None else chunks
    assert sum(chunks) == BSH * N // 128
    nchunk = len(chunks)
    nc = bass.Bass()
    x = nc.dram_tensor("x", [BSH, N, DIN], F32, kind="ExternalInput")
    w = nc.dram_tensor("W", [DIN, DOUT], F32, kind="ExternalInput")
    out = nc.dram_tensor("out", [BSH, DOUT], F32, kind="ExternalOutput")

    # (128, 128, 128): partition p, row-in-partition n, feature d
    x3 = x[:].flatten_outer_dims().rearrange("(p n) d -> p n d", p=128)
    starts = np.cumsum([0] + chunks).tolist()

    with ExitStack() as ctx:
        ec = ctx.enter_context
        xc = [ec(nc.sbuf_tensor(f"xc{c}", [128, chunks[c] * DIN], F32))
              for c in range(nchunk)]
        # bf16 fold buffers: first halving add writes here, then in-place
        red = [ec(nc.sbuf_tensor(f"red{c}", [128, max(chunks[c] // 2, 1) * DIN],
                                 BF16))
               for c in range(nchunk)]
        w_sb = ec(nc.sbuf_tensor("w_sb", [DIN, DOUT], F32))
        w_bf = ec(nc.sbuf_tensor("w_bf", [DIN, DOUT], BF16))
        mask_bf = ec(nc.sbuf_tensor("mask_bf", [128, BSH], BF16))
        s_bf = ec(nc.sbuf_tensor("s_bf", [DIN, BSH], BF16))
        out_sb = ec(nc.sbuf_tensor("out_sb", [BSH, DOUT], F32))
        psum_s = ec(nc.psum_tensor("psum_s", [DIN, BSH], F32))
        psum_o = ec(nc.psum_tensor("psum_o", [BSH, DOUT], F32))

        dma_w = ec(nc.semaphore("dma_w"))
        w_ready = ec(nc.semaphore("w_ready"))
        g_mask = ec(nc.semaphore("g_mask"))
        dma_c = [ec(nc.semaphore(f"dma_c{c}")) for c in range(nchunk)]
        v_red = ec(nc.semaphore("v_red"))    # +1 per finished red[c]
        pe_sem = ec(nc.semaphore("pe_sem"))
        v_sem = ec(nc.semaphore("v_sem"))    # s_bf ready
        v_out = ec(nc.semaphore("v_out"))
        dma_out = ec(nc.semaphore("dma_out"))
        block = ec(nc.Block())

        @block.sync
        def _(sync):
            for c in range(nchunk):
                sync.dma_start(
                    xc[c][:], x3[:, starts[c] : starts[c + 1], :]
                ).then_inc(dma_c[c], 16)
            sync.wait_ge(v_out, 1)
            sync.dma_start(out[:], out_sb[:]).then_inc(dma_out, 16)
            if wait_out:
                sync.wait_ge(dma_out, 16)

        @block.scalar
        def _(scalar):
            # W only feeds the final tiny matmul; the ACT HWDGE ring keeps it
            # off the sync ring, and Scalar does the bf16 cast itself.
            scalar.dma_start(w_sb[:], w[:]).then_inc(dma_w, 16)
            scalar.wait_ge(dma_w, 16)
            scalar.copy(w_bf[:], w_sb[:]).then_inc(w_ready, 1)

        @block.gpsimd
        def _(gpsimd):
            # 0/1 batch mask, one 32-partition quadrant at a time (nonzero
            # partition bases only allow 32-partition windows)
            op = None
            for q in range(4):
                for b in range(BSH):
                    op = gpsimd.memset(
                        mask_bf[32 * q : 32 * (q + 1), b : b + 1],
                        1.0 if q == b else 0.0,
                    )
            op.then_inc(g_mask, 1)

        @block.vector
        def _(vector):
            for c in range(nchunk):
                vector.wait_ge(dma_c[c], 16)
                rows = chunks[c]
                if rows == 1:
                    op = vector.tensor_copy(red[c][:, :DIN], xc[c][:, :DIN])
                else:
                    half = rows // 2 * DIN
                    # fp32 -> bf16 narrowing add, then 2x-rate bf16 halvings
                    op = vector.tensor_add(
                        red[c][:, :half], xc[c][:, :half],
                        xc[c][:, half : 2 * half],
                    )
                    s = half
                    while s > DIN:
                        s //= 2
                        op = vector.tensor_add(
                            red[c][:, :s], red[c][:, :s], red[c][:, s : 2 * s]
                        )
                op.then_inc(v_red, 1)
            vector.wait_ge(pe_sem, 1)
            vector.tensor_copy(s_bf[:], psum_s[:]).then_inc(v_sem, 1)
            vector.wait_ge(pe_sem, 2)
            vector.tensor_copy(out_sb[:], psum_o[:]).then_inc(v_out, 1)

        @block.tensor
        def _(tensor):
            tensor.wait_ge(g_mask, 1)
            # s[d, b] += sum_p red_c[p, d] * mask[p, b], accumulated over chunks
            for c in range(nchunk):
                tensor.wait_ge(v_red, c + 1)
                mm = tensor.matmul(
                    psum_s[:],
                    red[c][:, :DIN],
                    mask_bf[:],
                    start=(c == 0),
                    stop=(c == nchunk - 1),
                )
            mm.then_inc(pe_sem, 1)
            tensor.wait_ge(w_ready, 1)
            tensor.wait_ge(v_sem, 1)
            # out[b, jd] = sum_d s[d, b] * W[d, jd]
            tensor.matmul(
                psum_o[:], s_bf[:], w_bf[:], start=True, stop=True
            ).then_inc(pe_sem, 1)

    return nc


def _get_nc():
    if "nc" not in _cache:
        _cache["nc"] = _build_nc()
    return _cache["nc"]


def _in_maps(x, W):
    x = np.ascontiguousarray(x, dtype=np.float32)
    W = np.ascontiguousarray(W, dtype=np.float32)
    return [{"x": x[i * BSH : (i + 1) * BSH], "W": W} for i in range(N_CORES)]


def kernel(x, W, **profile_kwargs):
    nc = _get_nc()
    res = run_bass_kernel_spmd(nc, _in_maps(x, W), list(range(N_CORES)), **profile_kwargs)
    out = np.concatenate([r["out"] for r in res.results], axis=0)
    ret = out.reshape(B, 10, 16).astype(np.float32)
    if profile_kwargs:
        ret = (ret, res)
    return ret
